# revision 5
# baseline (speedup 1.0000x reference)
"""Trainium2 Bass kernel for nn_ASLRNN3 (self-contained).

Math (validated vs the jax reference):
  reference returns (outs[-1], hidden_final). outs[-1] depends only on the
  LAST frame of hand_data plus the hidden recurrence, and the recurrence
  h <- h @ h2h_w.T + h2h_b is input-independent. So:
    - device computes the per-frame branch for frame T-1 only (512 items)
    - host computes the recurrence exactly (h0 is zeros per the harness
      spec, making it a 32-step row-vector iteration)
    - out = relu(q_dev + (l2_b + h_T) @ out_w.T + out_b), where q_dev =
      i2h_pre @ out_w.T is linear in i2h, so channel-sharded partial i2h
      is pushed through out_w.T per-core and summed on the host.

Device decomposition (fully feature-major: features on partitions, all
512 items on the matmul free dim; every stage is a stationary-weight
bf16 matmul):
  conv1+pool2: p_T = relu(max(A_A.T@x_T, A_B.T@x_T) + bias)
  fc:          both_T = relu(Wfc.T @ p_T + bias), emitted as 3 row
               windows per hand aligned to the conv2/pool3 q-blocks
  conv2:       zA = sum_d D_cd.T @ both_d_window, where D_cd is a
               host-built 2-diagonal matrix carrying both conv taps
  pool3:       zB/zC = shifted-identity matmuls of zA; y2 =
               relu(max(zA,zB,zC) + bias)
  l2:          i2h_part_T += l2w_slice.T @ y2_T  (per-core channels)
  out:         q = out_w_fm.T @ i2h_part_T  ->  (10, 512) fp32 per core

Sharding: 8 cores, core j owns conv2 channels [4j, 4j+4). The per-item
hand stage is replicated (cheap); conv2/pool3/l2 are channel-sharded.
"""

import numpy as np

# ---------------------------------------------------------------- constants
T, B = 32, 512
HID, OUT = 500, 10
N = B  # items streamed on the free dim
NCORES = 8
CPC = 4  # conv2 channels per core

P0S = (0, 124, 248)     # pool3 output block starts (per channel)
WOUTS = (124, 124, 49)  # pool3 output block widths (sum 297)
WFCW = (127, 127, 52)   # fc window widths  (= Wo + 3)
W3OFF = (0, 127, 254)   # fc window col offsets within a (h,t) block
WFC_STRIDE = 306
C1LO = (0, 128, 256)
C1W = (128, 128, 48)
MBLK = 4                # i2h split into 4 x 125 rows
NWARM = 18              # PE warm-up matmuls (~3.5us to engage 2.4 GHz)

_prog = None


# ---------------------------------------------------------------- host packs
def _build_A_matrices(cw):
    """A_A/A_B for one hand: A[m, f] with m=x-feature (2j+d), f=o*19+i."""
    A_A = np.zeros((42, 304), np.float32)
    A_B = np.zeros((42, 304), np.float32)
    for o in range(16):
        for i in range(19):
            f = o * 19 + i
            for d in range(2):
                for kk in range(2):
                    A_A[2 * (i + kk) + d, f] += cw[o, d, kk]
                    A_B[2 * (i + 1 + kk) + d, f] += cw[o, d, kk]
    return A_A, A_B


def _host_pack(inputs):
    import ml_dtypes

    bf16 = ml_dtypes.bfloat16
    f32 = np.float32
    x = np.asarray(inputs["hand_data"], f32)[-1]  # (512, 84)
    xl = np.ascontiguousarray(x[:, :42].T).astype(bf16)  # (42, 512)
    xr = np.ascontiguousarray(x[:, 42:].T).astype(bf16)

    aal, abl = _build_A_matrices(np.asarray(inputs["l_conv_w"], f32))
    aar, abr = _build_A_matrices(np.asarray(inputs["r_conv_w"], f32))

    c1b = np.zeros((128, 6), f32)
    for h, cb in enumerate((inputs["l_conv_b"], inputs["r_conv_b"])):
        full = np.repeat(np.asarray(cb, f32), 19)  # (304,) bias per pooled feat
        for t in range(3):
            c1b[: C1W[t], h * 3 + t] = full[C1LO[t] : C1LO[t] + C1W[t]]

    wfc = np.zeros((128, 2 * 3 * WFC_STRIDE), f32)
    fcb = np.zeros((128, 6), f32)
    for h, (fwm, fb) in enumerate(
        ((inputs["l_fc_w"], inputs["l_fc_b"]), (inputs["r_fc_w"], inputs["r_fc_b"]))
    ):
        fwm = np.asarray(fwm, f32)  # (300, 304): [fout, fin]
        fb = np.asarray(fb, f32)
        for t in range(3):
            for qb in range(3):
                off = (h * 3 + t) * WFC_STRIDE + W3OFF[qb]
                blk = fwm[P0S[qb] : P0S[qb] + WFCW[qb],
                          C1LO[t] : C1LO[t] + C1W[t]].T
                wfc[: C1W[t], off : off + WFCW[qb]] = blk
        for qb in range(3):
            fcb[: WFCW[qb], h * 3 + qb] = fb[P0S[qb] : P0S[qb] + WFCW[qb]]

    ipad = np.zeros((128, 130), f32)
    for r in range(128):
        ipad[r, r + 2] = 1.0

    w2 = np.asarray(inputs["conv2_w"], f32)  # (32, 2, 2)
    b2 = np.asarray(inputs["conv2_b"], f32)
    l2_w = np.asarray(inputs["l2_w"], f32).reshape(HID, 32, 297)
    out_w = np.asarray(inputs["out_w"], f32)  # (10, 500)

    outw = np.zeros((125, 40), f32)
    for m in range(MBLK):
        outw[:, m * 10 : (m + 1) * 10] = out_w[:, m * 125 : (m + 1) * 125].T

    shared = {
        "xl": xl, "xr": xr,
        "aal": aal.astype(bf16), "abl": abl.astype(bf16),
        "aar": aar.astype(bf16), "abr": abr.astype(bf16),
        "c1b": c1b, "wfc": wfc.astype(bf16), "fcb": fcb,
        "ipad": ipad.astype(bf16), "outw": outw.astype(bf16),
    }

    in_maps = []
    for core in range(NCORES):
        dpk = np.zeros((128, 8, 132), f32)  # block c4*2+d: 2-diag D matrix
        b2c = np.zeros((128, CPC), f32)
        l2w = np.zeros((128, 12, HID), f32)
        for c4 in range(CPC):
            ch = core * CPC + c4
            for d in range(2):
                blk = dpk[:, c4 * 2 + d, :]
                for j in range(128):
                    blk[j, 2 + j] = w2[ch, d, 0]
                    if j + 1 < 128:
                        blk[j + 1, 2 + j] = w2[ch, d, 1]
            b2c[:, c4] = b2[ch]
            for qb in range(3):
                wo, p0 = WOUTS[qb], P0S[qb]
                l2w[:wo, c4 * 3 + qb, :] = l2_w[:, ch, p0 : p0 + wo].T
        m = dict(shared)
        m["dpk"] = dpk.astype(bf16)
        m["b2c"] = b2c
        m["l2w"] = l2w.astype(bf16)
        in_maps.append(m)
    return in_maps


def _host_const_and_hidden(inputs):
    f32 = np.float32
    hidden = np.asarray(inputs["hidden"], f32)
    W = np.asarray(inputs["h2h_w"], f32).T
    b = np.asarray(inputs["h2h_b"], f32)
    if np.any(hidden):
        h = hidden.copy()
        for _ in range(T):
            h = h @ W + b
        h_T = h
    else:
        s = np.zeros((HID,), f32)
        for _ in range(T):
            s = s @ W + b
        h_T = np.broadcast_to(s, (B, HID)).copy()
    const = (np.asarray(inputs["l2_b"], f32) + h_T) @ np.asarray(
        inputs["out_w"], f32
    ).T + np.asarray(inputs["out_b"], f32)
    return const, h_T


# ---------------------------------------------------------------- device prog
def _build_program():
    import concourse.mybir as mybir
    from concourse import bacc
    from concourse.tile import TileContext

    F32 = mybir.dt.float32
    BF16 = mybir.dt.bfloat16
    MAX = mybir.AluOpType.max
    ADD = mybir.AluOpType.add
    RELU = mybir.ActivationFunctionType.Relu
    COPY = mybir.ActivationFunctionType.Copy

    nc = bacc.Bacc("TRN2", target_bir_lowering=False, debug=False,
                   num_devices=NCORES)

    def din(name, shape, dt):
        return nc.dram_tensor(name, shape, dt, kind="ExternalInput").ap()

    xl_d = din("xl", [42, N], BF16)
    xr_d = din("xr", [42, N], BF16)
    aal_d = din("aal", [42, 304], BF16)
    abl_d = din("abl", [42, 304], BF16)
    aar_d = din("aar", [42, 304], BF16)
    abr_d = din("abr", [42, 304], BF16)
    c1b_d = din("c1b", [128, 6], F32)
    wfc_d = din("wfc", [128, 2 * 3 * WFC_STRIDE], BF16)
    fcb_d = din("fcb", [128, 6], F32)
    ipad_d = din("ipad", [128, 130], BF16)
    dpk_d = din("dpk", [128, 8, 132], BF16)
    b2c_d = din("b2c", [128, CPC], F32)
    l2w_d = din("l2w", [128, 12, HID], BF16)
    outw_d = din("outw", [125, 40], BF16)
    q_d = nc.dram_tensor("q", [OUT, N], F32, kind="ExternalOutput").ap()

    with TileContext(nc) as tc:
        with (
            tc.tile_pool(name="wp", bufs=1) as wp,       # persistent weights
            tc.tile_pool(name="act", bufs=1) as act,     # persistent activations
            tc.tile_pool(name="rot", bufs=4) as rot,     # rotating sbuf
            tc.tile_pool(name="ps_warm", bufs=1, space="PSUM") as ps_warm,
        ):
            # ---- PE warm-up: dense matmuls during the DMA phase to engage
            # the HAM clock gate (1.2 -> 2.4 GHz) before real work arrives.
            wtile = wp.tile([128, N], BF16, name="wtile")
            nc.gpsimd.memset(wtile, 0.0)
            wps = ps_warm.tile([128, N], F32, name="wps", bufs=1)
            for _ in range(NWARM):
                nc.tensor.matmul(wps, wtile[:, :128], wtile,
                                 start=True, stop=True)

            # ---- weight loads
            xl = wp.tile([42, N], BF16); nc.sync.dma_start(out=xl, in_=xl_d)
            xr = wp.tile([42, N], BF16); nc.sync.dma_start(out=xr, in_=xr_d)
            aal = wp.tile([42, 304], BF16); nc.sync.dma_start(out=aal, in_=aal_d)
            abl = wp.tile([42, 304], BF16); nc.sync.dma_start(out=abl, in_=abl_d)
            aar = wp.tile([42, 304], BF16); nc.sync.dma_start(out=aar, in_=aar_d)
            abr = wp.tile([42, 304], BF16); nc.sync.dma_start(out=abr, in_=abr_d)
            c1b = wp.tile([128, 6], F32); nc.sync.dma_start(out=c1b, in_=c1b_d)
            wfc = wp.tile([128, 2 * 3 * WFC_STRIDE], BF16)
            nc.sync.dma_start(out=wfc, in_=wfc_d)
            fcb = wp.tile([128, 6], F32); nc.sync.dma_start(out=fcb, in_=fcb_d)
            ipad = wp.tile([128, 130], BF16); nc.sync.dma_start(out=ipad, in_=ipad_d)
            dpk = wp.tile([128, 8, 132], BF16); nc.sync.dma_start(out=dpk, in_=dpk_d)
            b2c = wp.tile([128, CPC], F32); nc.sync.dma_start(out=b2c, in_=b2c_d)
            l2w = wp.tile([128, 12, HID], BF16); nc.sync.dma_start(out=l2w, in_=l2w_d)
            outw = wp.tile([125, 40], BF16); nc.sync.dma_start(out=outw, in_=outw_d)

            # ---- conv1 + pool2 + relu  ->  p_sb[h] tile [128, 3, N]
            p_sb = [None, None]
            with tc.tile_pool(name="ps_c1", bufs=1, space="PSUM") as ps_c1:
                for h, (xh, aa, ab) in enumerate(((xl, aal, abl), (xr, aar, abr))):
                    pt = act.tile([128, 3, N], BF16, tag=f"p_sb{h}",
                                  name=f"p_sb{h}")
                    p_sb[h] = pt
                    for t in range(3):
                        lo, w = C1LO[t], C1W[t]
                        pa = ps_c1.tile([128, N], F32, name="pa", bufs=2)
                        pb = ps_c1.tile([128, N], F32, name="pb", bufs=2)
                        nc.tensor.matmul(pa[:w], aa[:, lo : lo + w], xh,
                                         start=True, stop=True)
                        nc.tensor.matmul(pb[:w], ab[:, lo : lo + w], xh,
                                         start=True, stop=True)
                        pa_sb = rot.tile([128, N], BF16, tag="pa_sb",
                                         name="pa_sb")
                        nc.scalar.activation(pa_sb[:w], pa[:w], COPY)
                        u = rot.tile([128, N], BF16, tag="u_c1", name="u_c1")
                        nc.vector.tensor_tensor(out=u[:w], in0=pa_sb[:w],
                                                in1=pb[:w], op=MAX)
                        nc.vector.tensor_scalar(
                            out=pt[:w, t], in0=u[:w],
                            scalar1=c1b[:w, h * 3 + t : h * 3 + t + 1],
                            scalar2=0.0, op0=ADD, op1=MAX,
                        )

            # ---- fc -> both_sb[h][qb] (3 windows per hand)
            both_sb = [[None] * 3 for _ in range(2)]
            with tc.tile_pool(name="ps_fc", bufs=1, space="PSUM") as ps_fc:
                for h in range(2):
                    for qb in range(3):
                        ww, off = WFCW[qb], W3OFF[qb]
                        fcp = ps_fc.tile([128, N], F32, name="fcp", bufs=3)
                        for t in range(3):
                            base = (h * 3 + t) * WFC_STRIDE + off
                            nc.tensor.matmul(
                                fcp[:ww], wfc[: C1W[t], base : base + ww],
                                p_sb[h][: C1W[t], t],
                                start=(t == 0), stop=(t == 2),
                            )
                        bt = act.tile([128, N], BF16, tag=f"both{h}_{qb}",
                                      name=f"both{h}_{qb}")
                        both_sb[h][qb] = bt
                        nc.scalar.activation(
                            bt[:ww], fcp[:ww], RELU,
                            bias=fcb[:ww, h * 3 + qb : h * 3 + qb + 1],
                        )

            # ---- conv2 (2-diag D matmuls) + pool3 -> y2_sb[12]
            y2_sb = [None] * 12
            with (
                tc.tile_pool(name="ps_z", bufs=1, space="PSUM") as ps_z,
                tc.tile_pool(name="ps_sh", bufs=1, space="PSUM") as ps_sh,
            ):
                for c4 in range(CPC):
                    for qb in range(3):
                        wo = WOUTS[qb]
                        wiz = wo + 2
                        wfcw = WFCW[qb]
                        za = ps_z.tile([128, N], F32, name="za", bufs=2)
                        for d in range(2):
                            nc.tensor.matmul(
                                za[:wiz],
                                dpk[:wfcw, c4 * 2 + d, 2 : 2 + wiz],
                                both_sb[d][qb][:wfcw],
                                start=(d == 0), stop=(d == 1),
                            )
                        z_sb = rot.tile([128, N], BF16, tag="z_sb", name="z_sb")
                        nc.scalar.activation(z_sb[:wiz], za[:wiz], COPY)
                        zb = ps_sh.tile([128, N], F32, name="zb", bufs=2)
                        zc = ps_sh.tile([128, N], F32, name="zc", bufs=2)
                        nc.tensor.matmul(zb[:wo], ipad[:wiz, 3 : 3 + wo],
                                         z_sb[:wiz], start=True, stop=True)
                        nc.tensor.matmul(zc[:wo], ipad[:wiz, 4 : 4 + wo],
                                         z_sb[:wiz], start=True, stop=True)
                        t1 = rot.tile([128, N], BF16, tag="t1", name="t1")
                        nc.vector.tensor_tensor(out=t1[:wo], in0=z_sb[:wo],
                                                in1=zb[:wo], op=MAX)
                        u2 = rot.tile([128, N], BF16, tag="u2", name="u2")
                        nc.vector.tensor_tensor(out=u2[:wo], in0=t1[:wo],
                                                in1=zc[:wo], op=MAX)
                        yt = act.tile([128, N], BF16, tag=f"y2_{c4}_{qb}",
                                      name=f"y2_{c4}_{qb}")
                        y2_sb[c4 * 3 + qb] = yt
                        nc.vector.tensor_scalar(
                            out=yt[:wo], in0=u2[:wo],
                            scalar1=b2c[:wo, c4 : c4 + 1],
                            scalar2=0.0, op0=ADD, op1=MAX,
                        )

            # ---- l2 (48 accumulating matmuls) + i2h evac + out matmul
            with tc.tile_pool(name="ps_l2", bufs=1, space="PSUM") as ps_l2:
                i2h_ps = [ps_l2.tile([125, N], F32, tag=f"i2h{m}",
                                     name=f"i2h{m}") for m in range(MBLK)]
                nkt = CPC * 3
                for m in range(MBLK):
                    for kt in range(nkt):
                        wo = WOUTS[kt % 3]
                        nc.tensor.matmul(
                            i2h_ps[m],
                            l2w[:wo, kt, m * 125 : (m + 1) * 125],
                            y2_sb[kt][:wo],
                            start=(kt == 0), stop=(kt == nkt - 1),
                        )
                qp = ps_l2.tile([OUT, N], F32, tag="qp", name="qp")
                for m in range(MBLK):
                    ih = act.tile([125, N], BF16, tag="ih", bufs=2, name="ih")
                    nc.scalar.activation(ih, i2h_ps[m], COPY)
                    nc.tensor.matmul(qp, outw[:, m * 10 : (m + 1) * 10], ih,
                                     start=(m == 0), stop=(m == MBLK - 1))
                q_sb = act.tile([OUT, N], F32, tag="q_sb", name="q_sb")
                nc.vector.tensor_copy(out=q_sb, in_=qp)
                nc.sync.dma_start(out=q_d, in_=q_sb)

    nc.compile()
    return nc


def _get_program():
    global _prog
    if _prog is None:
        _prog = _build_program()
    return _prog


# ---------------------------------------------------------------- entry point
def kernel(**inputs):
    from concourse.bass_utils import run_bass_kernel_spmd

    nc = _get_program()
    in_maps = _host_pack(inputs)
    res = run_bass_kernel_spmd(nc, in_maps, core_ids=list(range(NCORES)))
    q_total = np.zeros((N, OUT), np.float32)
    for c in range(NCORES):
        q_total += res.results[c]["q"].T
    const, h_T = _host_const_and_hidden(inputs)
    out = np.maximum(q_total + const, 0.0).astype(np.float32)
    return out, h_T.astype(np.float32)


# revision 12
# speedup vs baseline: 1.7477x; 1.7477x over previous
"""Trainium2 Bass kernel for nn_ASLRNN3 (self-contained).

Math (validated vs the jax reference):
  reference returns (outs[-1], hidden_final). outs[-1] depends only on the
  LAST frame of hand_data plus the hidden recurrence, and the recurrence
  h <- h @ h2h_w.T + h2h_b is input-independent. So:
    - device computes the per-frame branch for frame T-1 only (512 items)
    - host computes the recurrence exactly (h0 is zeros per the harness
      spec, making it a 32-step row-vector iteration)
    - out = relu(q_dev + (l2_b + h_T) @ out_w.T + out_b), where q_dev =
      i2h_pre @ out_w.T is linear in i2h, so channel-sharded partial i2h
      is pushed through out_w.T per-core and summed on the host.

Device decomposition (fully feature-major: features on partitions, all
512 items on the matmul free dim; every stage is a stationary-weight
bf16 matmul):
  conv1+pool2: p_T = relu(max(A_A.T@x_T, A_B.T@x_T) + bias)
  fc:          both_T = relu(Wfc.T @ p_T + bias), emitted as 3 row
               windows per hand aligned to the conv2/pool3 q-blocks
  conv2:       zA = sum_d D_cd.T @ both_d_window, where D_cd is a
               host-built 2-diagonal matrix carrying both conv taps
  pool3:       zB/zC = shifted-identity matmuls of zA; y2 =
               relu(max(zA,zB,zC) + bias)
  l2:          i2h_part_T += l2w_slice.T @ y2_T  (per-core channels)
  out:         q = out_w_fm.T @ i2h_part_T  ->  (10, 512) fp32 per core

Sharding: 8 cores, core j owns conv2 channels [4j, 4j+4). The per-item
hand stage is replicated (cheap); conv2/pool3/l2 are channel-sharded.
"""

import numpy as np

# ---------------------------------------------------------------- constants
T, B = 32, 512
HID, OUT = 500, 10
N = B  # items streamed on the free dim
NCORES = 8
CPC = 4  # conv2 channels per core

P0S = (0, 124, 248)     # pool3 output block starts (per channel)
WOUTS = (124, 124, 49)  # pool3 output block widths (sum 297)
WFCW = (127, 127, 52)   # fc window widths  (= Wo + 3)
WW3 = (128, 128, 53)    # fc window rows incl ones row (= WFCW + 1)
W3OFF = (0, 128, 256)   # fc window col offsets within a (h,t) block
WFC_STRIDE = 309
C1LO = (0, 128, 256)
C1W = (128, 128, 48)
MBLK = 4                # i2h split into 4 x 125 rows
NWARM = 18              # PE warm-up matmuls (~3.5us to engage 2.4 GHz)

_prog = None


# ---------------------------------------------------------------- host packs
def _build_A_matrices(cw):
    """A_A/A_B for one hand: A[m, f] with m=x-feature (2j+d), f=o*19+i."""
    A_A = np.zeros((42, 304), np.float32)
    A_B = np.zeros((42, 304), np.float32)
    for o in range(16):
        for i in range(19):
            f = o * 19 + i
            for d in range(2):
                for kk in range(2):
                    A_A[2 * (i + kk) + d, f] += cw[o, d, kk]
                    A_B[2 * (i + 1 + kk) + d, f] += cw[o, d, kk]
    return A_A, A_B


def _host_pack(inputs):
    import ml_dtypes

    bf16 = ml_dtypes.bfloat16
    f32 = np.float32
    x = np.asarray(inputs["hand_data"], f32)[-1]  # (512, 84)
    ones = np.ones((1, x.shape[0]), f32)
    xl = np.concatenate([x[:, :42].T, ones], axis=0).astype(bf16)  # (43, 512)
    xr = np.concatenate([x[:, 42:].T, ones], axis=0).astype(bf16)

    # A matrices carry the conv1 bias on the ones row (row 42)
    def hand_A(cw_key, cb_key):
        A_A, A_B = _build_A_matrices(np.asarray(inputs[cw_key], f32))
        brow = np.repeat(np.asarray(inputs[cb_key], f32), 19)[None, :]  # (1,304)
        return (np.concatenate([A_A, brow], axis=0),
                np.concatenate([A_B, brow], axis=0))

    aal, abl = hand_A("l_conv_w", "l_conv_b")
    aar, abr = hand_A("r_conv_w", "r_conv_b")

    # fc windows: data rows 0..WFCW-1, plus an always-one output at row WFCW
    # (zero weight column + bias 1.0) used to inject the conv2 bias.
    wfc = np.zeros((128, 2 * 3 * WFC_STRIDE), f32)
    fcb = np.zeros((128, 6), f32)
    for h, (fwm, fb) in enumerate(
        ((inputs["l_fc_w"], inputs["l_fc_b"]), (inputs["r_fc_w"], inputs["r_fc_b"]))
    ):
        fwm = np.asarray(fwm, f32)  # (300, 304): [fout, fin]
        fb = np.asarray(fb, f32)
        for t in range(3):
            for qb in range(3):
                off = (h * 3 + t) * WFC_STRIDE + W3OFF[qb]
                blk = fwm[P0S[qb] : P0S[qb] + WFCW[qb],
                          C1LO[t] : C1LO[t] + C1W[t]].T
                wfc[: C1W[t], off : off + WFCW[qb]] = blk
        for qb in range(3):
            fcb[: WFCW[qb], h * 3 + qb] = fb[P0S[qb] : P0S[qb] + WFCW[qb]]
            fcb[WFCW[qb], h * 3 + qb] = 1.0

    w2 = np.asarray(inputs["conv2_w"], f32)  # (32, 2, 2)
    b2 = np.asarray(inputs["conv2_b"], f32)
    l2_w = np.asarray(inputs["l2_w"], f32).reshape(HID, 32, 297)
    out_w = np.asarray(inputs["out_w"], f32)  # (10, 500)

    outw = np.zeros((125, 40), f32)
    for m in range(MBLK):
        outw[:, m * 10 : (m + 1) * 10] = out_w[:, m * 125 : (m + 1) * 125].T

    shared = {
        "xl": xl, "xr": xr,
        "aal": aal.astype(bf16), "abl": abl.astype(bf16),
        "aar": aar.astype(bf16), "abr": abr.astype(bf16),
        "wfc": wfc.astype(bf16), "fcb": fcb,
        "outw": outw.astype(bf16),
    }

    in_maps = []
    for core in range(NCORES):
        # dpk block (c4*3+qb)*2+d: 2-diagonal conv2 taps; conv2 bias on the
        # ones row (row WFCW[qb]) of the d=0 matrix.
        dpk = np.zeros((128, 24, 132), f32)
        l2w = np.zeros((128, 12, HID), f32)
        for c4 in range(CPC):
            ch = core * CPC + c4
            for qb in range(3):
                wo, p0, wfcw = WOUTS[qb], P0S[qb], WFCW[qb]
                for d in range(2):
                    blk = dpk[:, (c4 * 3 + qb) * 2 + d, :]
                    for cc in range(2, 2 + wo + 2):
                        blk[cc - 2, cc] = w2[ch, d, 0]
                        blk[cc - 1, cc] = w2[ch, d, 1]
                    if d == 0:
                        blk[wfcw, 2 : 2 + wo + 2] = b2[ch]
                l2w[:wo, c4 * 3 + qb, :] = l2_w[:, ch, p0 : p0 + wo].T
        m = dict(shared)
        m["dpk"] = dpk.astype(bf16)
        m["l2w"] = l2w.astype(bf16)
        in_maps.append(m)
    return in_maps


def _host_const_and_hidden(inputs):
    f32 = np.float32
    hidden = np.asarray(inputs["hidden"], f32)
    W = np.asarray(inputs["h2h_w"], f32).T
    b = np.asarray(inputs["h2h_b"], f32)
    if np.any(hidden):
        h = hidden.copy()
        for _ in range(T):
            h = h @ W + b
        h_T = h
    else:
        s = np.zeros((HID,), f32)
        for _ in range(T):
            s = s @ W + b
        h_T = np.broadcast_to(s, (B, HID)).copy()
    const = (np.asarray(inputs["l2_b"], f32) + h_T) @ np.asarray(
        inputs["out_w"], f32
    ).T + np.asarray(inputs["out_b"], f32)
    return const, h_T


# ---------------------------------------------------------------- device prog
def _build_program():
    import concourse.mybir as mybir
    from concourse import bacc
    from concourse.tile import TileContext

    F32 = mybir.dt.float32
    BF16 = mybir.dt.bfloat16
    MAX = mybir.AluOpType.max
    ADD = mybir.AluOpType.add
    RELU = mybir.ActivationFunctionType.Relu
    COPY = mybir.ActivationFunctionType.Copy

    nc = bacc.Bacc("TRN2", target_bir_lowering=False, debug=False,
                   num_devices=NCORES)

    def din(name, shape, dt):
        return nc.dram_tensor(name, shape, dt, kind="ExternalInput").ap()

    xl_d = din("xl", [43, N], BF16)
    xr_d = din("xr", [43, N], BF16)
    aal_d = din("aal", [43, 304], BF16)
    abl_d = din("abl", [43, 304], BF16)
    aar_d = din("aar", [43, 304], BF16)
    abr_d = din("abr", [43, 304], BF16)
    wfc_d = din("wfc", [128, 2 * 3 * WFC_STRIDE], BF16)
    fcb_d = din("fcb", [128, 6], F32)
    dpk_d = din("dpk", [128, 24, 132], BF16)
    l2w_d = din("l2w", [128, 12, HID], BF16)
    outw_d = din("outw", [125, 40], BF16)
    q_d = nc.dram_tensor("q", [OUT, N], F32, kind="ExternalOutput").ap()

    with TileContext(nc) as tc:
        with (
            tc.tile_pool(name="wp", bufs=1) as wp,       # persistent weights
            tc.tile_pool(name="act", bufs=1) as act,     # persistent activations
            tc.tile_pool(name="rot", bufs=4) as rot,     # rotating sbuf
        ):
            # ---- PE warm-up: dense matmuls during the DMA phase to engage
            # the HAM clock gate (1.2 -> 2.4 GHz) before real work arrives.
            wtile = wp.tile([128, N], BF16, name="wtile")
            nc.gpsimd.memset(wtile, 0.0)
            with tc.tile_pool(name="ps_warm", bufs=1, space="PSUM") as ps_warm:
                wps = ps_warm.tile([128, N], F32, name="wps", bufs=1)
                for _ in range(NWARM):
                    nc.tensor.matmul(wps, wtile[:, :128], wtile,
                                     start=True, stop=True)

            # ---- weight loads
            xl = wp.tile([43, N], BF16); nc.sync.dma_start(out=xl, in_=xl_d)
            xr = wp.tile([43, N], BF16); nc.sync.dma_start(out=xr, in_=xr_d)
            aal = wp.tile([43, 304], BF16); nc.sync.dma_start(out=aal, in_=aal_d)
            abl = wp.tile([43, 304], BF16); nc.sync.dma_start(out=abl, in_=abl_d)
            aar = wp.tile([43, 304], BF16); nc.sync.dma_start(out=aar, in_=aar_d)
            abr = wp.tile([43, 304], BF16); nc.sync.dma_start(out=abr, in_=abr_d)
            wfc = wp.tile([128, 2 * 3 * WFC_STRIDE], BF16)
            nc.sync.dma_start(out=wfc, in_=wfc_d)
            fcb = wp.tile([128, 6], F32); nc.sync.dma_start(out=fcb, in_=fcb_d)
            dpk = wp.tile([128, 24, 132], BF16); nc.sync.dma_start(out=dpk, in_=dpk_d)
            l2w = wp.tile([128, 12, HID], BF16); nc.sync.dma_start(out=l2w, in_=l2w_d)
            outw = wp.tile([125, 40], BF16); nc.sync.dma_start(out=outw, in_=outw_d)

            # ---- conv1 + pool2 + relu  ->  p_sb[h] tile [128, 3, N]
            p_sb = [None, None]
            with tc.tile_pool(name="ps_c1", bufs=1, space="PSUM") as ps_c1:
                for h, (xh, aa, ab) in enumerate(((xl, aal, abl), (xr, aar, abr))):
                    pt = act.tile([128, 3, N], BF16, tag=f"p_sb{h}",
                                  name=f"p_sb{h}")
                    p_sb[h] = pt
                    for t in range(3):
                        lo, w = C1LO[t], C1W[t]
                        pa = ps_c1.tile([128, N], F32, name="pa", bufs=2)
                        pb = ps_c1.tile([128, N], F32, name="pb", bufs=2)
                        nc.tensor.matmul(pa[:w], aa[:, lo : lo + w], xh,
                                         start=True, stop=True)
                        nc.tensor.matmul(pb[:w], ab[:, lo : lo + w], xh,
                                         start=True, stop=True)
                        # p = relu(max(pa,pb)) = max(relu(pa), pb)
                        u = rot.tile([128, N], BF16, tag="u_c1", name="u_c1")
                        nc.scalar.activation(u[:w], pa[:w], RELU)
                        nc.vector.tensor_tensor(out=pt[:w, t], in0=u[:w],
                                                in1=pb[:w], op=MAX)

            # ---- fc -> both_sb[h][qb] (3 windows per hand)
            both_sb = [[None] * 3 for _ in range(2)]
            with tc.tile_pool(name="ps_fc", bufs=1, space="PSUM") as ps_fc:
                for h in range(2):
                    for qb in range(3):
                        ww, off = WW3[qb], W3OFF[qb]
                        fcp = ps_fc.tile([128, N], F32, name="fcp", bufs=3)
                        for t in range(3):
                            base = (h * 3 + t) * WFC_STRIDE + off
                            nc.tensor.matmul(
                                fcp[:ww], wfc[: C1W[t], base : base + ww],
                                p_sb[h][: C1W[t], t],
                                start=(t == 0), stop=(t == 2),
                            )
                        bt = act.tile([128, N], BF16, tag=f"both{h}_{qb}",
                                      name=f"both{h}_{qb}")
                        both_sb[h][qb] = bt
                        nc.scalar.activation(
                            bt[:ww], fcp[:ww], RELU,
                            bias=fcb[:ww, h * 3 + qb : h * 3 + qb + 1],
                        )
                        nc.tensor.matmul(wps, wtile[:, :128], wtile,
                                          start=True, stop=True)

            # ---- conv2: zA/zB/zC directly via col-shifted 2-diag D matmuls
            # (bias baked in via the fc ones row); pool3+relu as a max chain
            # max(relu(zA), zB, zC) = relu(max3(z)); l2 matmuls interleaved.
            with (
                tc.tile_pool(name="ps_i2h", bufs=1, space="PSUM") as ps_i2h,
                tc.tile_pool(name="ps_z", bufs=1, space="PSUM") as ps_z,
            ):
                i2h_ps = [ps_i2h.tile([125, N], F32, tag=f"i2h{m}",
                                      name=f"i2h{m}") for m in range(MBLK)]
                nkt = CPC * 3
                for c4 in range(CPC):
                    for qb in range(3):
                        kt = c4 * 3 + qb
                        wo = WOUTS[qb]
                        ww = WW3[qb]
                        za = ps_z.tile([128, N], F32, name="za", bufs=2)
                        zb = ps_z.tile([128, N], F32, name="zb", bufs=1)
                        zc = ps_z.tile([128, N], F32, name="zc", bufs=1)
                        for s, zt in enumerate((za, zb, zc)):
                            for d in range(2):
                                nc.tensor.matmul(
                                    zt[:wo],
                                    dpk[:ww, kt * 2 + d, 2 + s : 2 + s + wo],
                                    both_sb[d][qb][:ww],
                                    start=(d == 0), stop=(d == 1),
                                )
                        s1 = rot.tile([128, N], BF16, tag="s1", name="s1")
                        nc.scalar.activation(s1[:wo], za[:wo], RELU)
                        s2 = rot.tile([128, N], BF16, tag="s2", name="s2")
                        nc.vector.tensor_tensor(out=s2[:wo], in0=s1[:wo],
                                                in1=zb[:wo], op=MAX)
                        yt = act.tile([128, N], BF16, tag=f"y2_{kt}",
                                      name=f"y2_{kt}")
                        nc.vector.tensor_tensor(out=yt[:wo], in0=s2[:wo],
                                                in1=zc[:wo], op=MAX)
                        for m in range(MBLK):
                            nc.tensor.matmul(
                                i2h_ps[m],
                                l2w[:wo, kt, m * 125 : (m + 1) * 125],
                                yt[:wo],
                                start=(kt == 0), stop=(kt == nkt - 1),
                            )

                # ---- i2h evac + out matmul
                with tc.tile_pool(name="ps_q", bufs=1, space="PSUM") as ps_q:
                    qp = ps_q.tile([OUT, N], F32, tag="qp", name="qp")
                    for m in range(MBLK):
                        ih = act.tile([125, N], BF16, tag="ih", bufs=2,
                                      name="ih")
                        nc.scalar.activation(ih, i2h_ps[m], COPY)
                        nc.tensor.matmul(qp, outw[:, m * 10 : (m + 1) * 10],
                                         ih, start=(m == 0),
                                         stop=(m == MBLK - 1))
                    q_sb = act.tile([OUT, N], F32, tag="q_sb", name="q_sb")
                    nc.vector.tensor_copy(out=q_sb, in_=qp)
                    nc.sync.dma_start(out=q_d, in_=q_sb)

    nc.compile()
    return nc


def _get_program():
    global _prog
    if _prog is None:
        _prog = _build_program()
    return _prog


# ---------------------------------------------------------------- entry point
def kernel(**inputs):
    from concourse.bass_utils import run_bass_kernel_spmd

    nc = _get_program()
    in_maps = _host_pack(inputs)
    res = run_bass_kernel_spmd(nc, in_maps, core_ids=list(range(NCORES)))
    q_total = np.zeros((N, OUT), np.float32)
    for c in range(NCORES):
        q_total += res.results[c]["q"].T
    const, h_T = _host_const_and_hidden(inputs)
    out = np.maximum(q_total + const, 0.0).astype(np.float32)
    return out, h_T.astype(np.float32)
